# revision 41
# baseline (speedup 1.0000x reference)
"""Trainium2 Bass kernel for nn_CNN3_P (dense_cnn), 8-core data parallel.

Network (per sample):
  x [128,64] -> pairwise conv -> relu -> [256,127]
  -> conv1d k3 (x3, relu) -> [256,121] -> FC 30976->512 relu -> FC 512->1

Strategy: batch 2048 split 256/core. Channels on partitions (2 chunks of
128); all layers run on a flat [128, T*128] layout (stride 128 per
sample). Conv layers compute only the valid positions (125/123/121 per
sample, offset 1) via 3D moving APs -- the K=3 shifts are column
offsets within each sample's 128-col block. All matmuls in fp16 (1
cyc/col on the PE), PSUM accumulates fp32. Conv3 output is stored
(s, l)-major so FC1's stationary operands are contiguous; Wf1 streams
through SBUF once (first two chunks prefetched at kernel start so the
FC phase starts without a DMA stall). FC2 is a single DVE
tensor_tensor_reduce per 128-sample chunk (no PE transposes).
"""
import os
import sys

for _p in ('/opt/trn_rl_repo', '/root/.axon_site/_ro/trn_rl_repo'):
    if os.path.isdir(_p) and _p not in sys.path:
        sys.path.insert(0, _p)

import numpy as np
import ml_dtypes

import concourse.bacc as bacc
import concourse.mybir as mybir
import concourse.tile as tile
from concourse.bass_utils import run_bass_kernel_spmd
from concourse.masks import make_identity

F32 = mybir.dt.float32
F16 = mybir.dt.float16

P = 128
CL = 128          # context length
IL = 64           # inst length
PC = 256          # channels (all layers)
NCHUNK = 2        # channel chunks of 128
LF = 121          # conv3 valid positions
F1 = 512
N_CORES = 8
B = 2048
BCORE = B // N_CORES      # 256
T = 8                     # samples per conv sub-tile
NT = BCORE // T           # 32
FLAT = T * CL             # 1024
TILE_N = 512              # psum tile width (4 samples * 128)
NTC = FLAT // TILE_N      # 2
SPT = TILE_N // CL        # samples per psum tile (4)
SC = BCORE // P           # 2 sample chunks of 128 for FC
GL = 11                   # l-slices per Wf1 DMA (121 = 11*11)
CONV_L = [125, 123, 121]  # valid output positions per conv layer


def build_nc():
    nc = bacc.Bacc("TRN2", target_bir_lowering=False, debug=False)

    xt_d = nc.dram_tensor("xth", [IL, BCORE, CL], F16, kind="ExternalInput")
    xb_d = nc.dram_tensor("xbh", [IL, BCORE, CL], F16, kind="ExternalInput")
    wpc_d = nc.dram_tensor("wpc", [P, PC], F16, kind="ExternalInput")
    bp_d = nc.dram_tensor("bpc", [NCHUNK, P], F32, kind="ExternalInput")
    wc_d = [nc.dram_tensor(f"w{i}t", [NCHUNK, 3, NCHUNK, P, P], F16,
                           kind="ExternalInput") for i in (1, 2, 3)]
    bc_d = [nc.dram_tensor(f"b{i}c", [NCHUNK, P], F32, kind="ExternalInput")
            for i in (1, 2, 3)]
    wf1_d = nc.dram_tensor("wf1t", [NCHUNK, LF // GL, P, GL, F1], F16,
                           kind="ExternalInput")
    bf1_d = nc.dram_tensor("bf1r", [1, F1], F16, kind="ExternalInput")
    ones_d = nc.dram_tensor("onesr", [1, P], F16, kind="ExternalInput")
    wf2_d = nc.dram_tensor("wf2rep", [P, F1], F16, kind="ExternalInput")
    bf2_d = nc.dram_tensor("bf2col", [P, 1], F32, kind="ExternalInput")
    y_d = nc.dram_tensor("y", [BCORE, 1], F32, kind="ExternalOutput")

    RELU = mybir.ActivationFunctionType.Relu

    with tile.TileContext(nc) as tc:
        with tc.tile_pool(name="const", bufs=1) as cpool, \
             tc.tile_pool(name="h3c", bufs=1) as h3pool, \
             tc.tile_pool(name="xt", bufs=2) as xtpool, \
             tc.tile_pool(name="fps", bufs=2, space="PSUM") as fpspool:
            # --- constants / weights, resident all kernel ---
            wpc = cpool.tile([P, PC], F16)
            nc.sync.dma_start(wpc[:], wpc_d.ap())
            bp = cpool.tile([P, NCHUNK], F32)
            nc.sync.dma_start(bp[:], bp_d.ap().rearrange("c p -> p c"))
            # tile-0 input DMAs go first on the sync queue so the first
            # pairwise matmul isn't gated on the gpsimd queue boot; the
            # nt=1 half (samples 4-7, consumed first) is DMA'd first so
            # the first matmul starts as soon as that half lands
            xt0 = xtpool.tile([P, T * CL], F16, tag="xt", name="xt")
            HT = T // 2
            for half in (1, 0):
                hs = slice(half * HT, (half + 1) * HT)
                cs = slice(half * HT * CL, (half + 1) * HT * CL)
                nc.sync.dma_start(
                    xt0[0:IL, cs].rearrange("p (s i) -> p s i", i=CL),
                    xt_d.ap()[:, hs, :])
                nc.sync.dma_start(
                    xt0[IL:P, cs].rearrange("p (s i) -> p s i", i=CL),
                    xb_d.ap()[:, hs, :])
            # conv weights: per layer, per ci-chunk: [ci, (k, coc, co)]
            wconv = []
            for i in range(3):
                tiles = []
                for cic in range(NCHUNK):
                    w = cpool.tile([P, 3 * NCHUNK * P], F16, tag=f"w{i}_{cic}")
                    nc.sync.dma_start(
                        w[:].rearrange("p (k b c) -> p k b c", k=3, b=NCHUNK),
                        wc_d[i].ap()[cic].rearrange("k b p c -> p k b c"))
                    tiles.append(w)
                wconv.append(tiles)
            bconv = []
            for i in range(3):
                bt = cpool.tile([P, NCHUNK], F32, tag=f"bc{i}")
                nc.sync.dma_start(bt[:], bc_d[i].ap().rearrange("c p -> p c"))
                bconv.append(bt)
            bf1 = cpool.tile([1, F1], F16)
            nc.sync.dma_start(bf1[:], bf1_d.ap())
            ones = cpool.tile([1, P], F16)
            nc.sync.dma_start(ones[:], ones_d.ap())
            wf2rep = cpool.tile([P, F1], F16)
            nc.sync.dma_start(wf2rep[:], wf2_d.ap())
            bf2col = cpool.tile([P, 1], F32)
            nc.sync.dma_start(bf2col[:], bf2_d.ap())
            # prefetch the first two Wf1 chunks so the FC phase starts hot
            wf1pre = []
            for i in range(1):
                rw = cpool.tile([P, GL * F1], F16, tag=f"wf1p{i}")
                nc.sync.dma_start(rw[:].rearrange("p (l f) -> p l f", l=GL),
                                  wf1_d.ap()[0, i])
                wf1pre.append(rw)
            # f32 identity for the tiny y transposes at the end (built on
            # gpsimd, which is otherwise idle until conv tile 1)
            ident = cpool.tile([P, P], F32)
            make_identity(nc, ident[:])
            # HAM warmup: the PE is idle from engine boot (~6us) until the
            # first input DMA lands (~12us); burn that window with dummy
            # matmuls so the clock gate is at 8/8 when real work arrives
            wdum = cpool.tile([P, TILE_N], F16, tag="wdum")
            nc.vector.memset(wdum[:], 0.0)

            # persistent conv3 output, fp16, (l, s)-major: col = l*BCORE + s,
            # so FC1's stationary slices are contiguous and get Fast Weight
            # Load (non-contiguous LDWEIGHTS at ~210ns barely hides under
            # the 216ns matmuls)
            h3c = [h3pool.tile([P, (LF + 1) * BCORE], F16, tag=f"h3c{cc}",
                               name=f"h3c{cc}") for cc in range(NCHUNK)]
            h3v = [h.rearrange("p (l s) -> p l s", s=BCORE) for h in h3c]

            # ---------------- conv phase ----------------
            with tc.tile_pool(name="h", bufs=2) as hpool, \
                 tc.tile_pool(name="ps", bufs=6, space="PSUM") as pspool:
                NTS = list(range(NTC - 1, -1, -1))   # nt=1 first: its consumers
                # don't cross the nt boundary, so they unblock earliest

                for _ in range(9):
                    psd = pspool.tile([P, TILE_N], F32, tag="ps", name="warm")
                    nc.tensor.matmul(psd[:], wdum[:, 0:P], wdum[:],
                                     start=True, stop=True)

                def pairwise(t):
                    if t == 0:
                        xt = xt0
                    else:
                        xt = xtpool.tile([P, T * CL], F16, tag="xt", name="xt")
                        nc.gpsimd.dma_start(
                            xt[0:IL, :].rearrange("p (s i) -> p s i", i=CL),
                            xt_d.ap()[:, t * T:(t + 1) * T, :])
                        nc.gpsimd.dma_start(
                            xt[IL:P, :].rearrange("p (s i) -> p s i", i=CL),
                            xb_d.ap()[:, t * T:(t + 1) * T, :])
                    xv = xt.rearrange("p (s i) -> p s i", i=CL)
                    h0 = [hpool.tile([P, FLAT], F16, tag=f"h0_{cc}", name=f"h0_{cc}")
                          for cc in range(NCHUNK)]
                    h0v = [h.rearrange("p (s i) -> p s i", i=CL) for h in h0]
                    LP = CL - 1   # col 0 is never consumed by conv1
                    for nt in NTS:
                        for cc in range(NCHUNK):
                            ps = pspool.tile([P, TILE_N], F32, tag="ps", name="pwps")
                            ss = slice(nt * SPT, (nt + 1) * SPT)
                            nc.tensor.matmul(ps[:, 0:SPT * LP],
                                             wpc[:, cc * P:(cc + 1) * P],
                                             xv[:, ss, 1:1 + LP],
                                             start=True, stop=True)
                            nc.scalar.activation(
                                h0v[cc][:, ss, 1:1 + LP],
                                ps[:, 0:SPT * LP].rearrange("p (s l) -> p s l", l=LP),
                                RELU, bias=bp[:, cc:cc + 1])
                    return h0

                def conv_layer(hin, w_tiles, L, evac):
                    # compute only the L valid positions (offset 1) per
                    # sample via 3D APs; group-outer so each psum group
                    # completes early and its evacuation overlaps the
                    # remaining groups' matmuls
                    hv = [h.rearrange("p (s l) -> p s l", l=CL) for h in hin]
                    for nt in NTS:
                        for co in range(NCHUNK):
                            ps = pspool.tile([P, TILE_N], F32,
                                             tag="ps", name=f"cps{co}_{nt}")
                            psf = ps[:, 0:SPT * L]
                            step = 0
                            for k in range(3):
                                for ci in range(NCHUNK):
                                    lhsT = w_tiles[ci][:, (k * NCHUNK + co) * P:
                                                       (k * NCHUNK + co + 1) * P]
                                    rhs = hv[ci][:, nt * SPT:(nt + 1) * SPT,
                                                 1 + k:1 + k + L]
                                    nc.tensor.matmul(psf, lhsT, rhs,
                                                     start=(step == 0),
                                                     stop=(step == 5))
                                    step += 1
                            evac(co, nt, psf.rearrange("p (s l) -> p s l", l=L))

                h0_next = pairwise(0)
                for t in range(NT):
                    h0 = h0_next
                    h1 = [hpool.tile([P, FLAT], F16, tag=f"h1_{cc}", name=f"h1_{cc}")
                          for cc in range(NCHUNK)]
                    h1v = [h.rearrange("p (s l) -> p s l", l=CL) for h in h1]

                    def evac1(co, nt, psv, L=CONV_L[0]):
                        nc.vector.tensor_scalar(
                            h1v[co][:, nt * SPT:(nt + 1) * SPT, 1:1 + L], psv,
                            bconv[0][:, co:co + 1], 0.0,
                            mybir.AluOpType.add, mybir.AluOpType.max)
                    conv_layer(h0, wconv[0], CONV_L[0], evac1)

                    # emit next tile's pairwise here so its evacuations age
                    # a full tile before conv1(t+1) consumes them
                    if t + 1 < NT:
                        h0_next = pairwise(t + 1)

                    h2 = [hpool.tile([P, FLAT], F16, tag=f"h2_{cc}", name=f"h2_{cc}")
                          for cc in range(NCHUNK)]
                    h2v = [h.rearrange("p (s l) -> p s l", l=CL) for h in h2]

                    def evac2(co, nt, psv, L=CONV_L[1]):
                        nc.vector.tensor_scalar(
                            h2v[co][:, nt * SPT:(nt + 1) * SPT, 1:1 + L], psv,
                            bconv[1][:, co:co + 1], 0.0,
                            mybir.AluOpType.add, mybir.AluOpType.max)
                    conv_layer(h1, wconv[1], CONV_L[1], evac2)

                    def evac3(co, nt, psv, t=t, L=CONV_L[2]):
                        # psum is (s, l)-major; iterate both sides l-outer,
                        # s-inner so the write lands in the (l, s) layout
                        s0 = t * T + nt * SPT
                        pst = psv.rearrange("p s l -> p l s")
                        nc.scalar.activation(h3v[co][:, 1:1 + L, s0:s0 + SPT],
                                             pst, RELU, bias=bconv[2][:, co:co + 1])
                    conv_layer(h2, wconv[2], CONV_L[2], evac3)

            # ---------------- FC phase ----------------
            with tc.tile_pool(name="wf1", bufs=4) as wfpool, \
                 tc.tile_pool(name="h4", bufs=1) as h4pool, \
                 tc.tile_pool(name="yps", bufs=1, space="PSUM") as ypspool:
                ps_fc1 = [fpspool.tile([P, F1], F32, tag=f"fc1ps{sc}", bufs=1,
                                       name=f"fc1ps{sc}") for sc in range(SC)]
                for sc in range(SC):
                    nc.tensor.matmul(ps_fc1[sc][:], ones[:], bf1[:],
                                     start=True, stop=False)
                NG = LF // GL
                rw_tiles = {}

                def get_rw(cc, lg):
                    if cc == 0 and lg < 1:
                        return wf1pre[lg]
                    rw = wfpool.tile([P, GL * F1], F16, tag="wf1")
                    nc.sync.dma_start(rw[:].rearrange("p (l f) -> p l f", l=GL),
                                      wf1_d.ap()[cc, lg])
                    return rw

                def fc1_mm(cc, lg, ll, sc, rw):
                    l = lg * GL + ll
                    last = (cc == NCHUNK - 1) and (l == LF - 1)
                    # valid conv3 position l sits at flat l+1
                    nc.tensor.matmul(
                        ps_fc1[sc][:],
                        h3v[cc][:, l + 1, sc * P:(sc + 1) * P],
                        rw[:, ll * F1:(ll + 1) * F1],
                        start=False, stop=last)

                for cc in range(NCHUNK):
                    for lg in range(NG - 2 if cc == NCHUNK - 1 else NG):
                        rw = get_rw(cc, lg)
                        for ll in range(GL):
                            for sc in range(SC):
                                fc1_mm(cc, lg, ll, sc, rw)
                # last two groups sc-outer: chunk 0 finishes 2*GL matmuls
                # early so its whole FC2 chain hides under chunk 1's tail
                rwa = get_rw(NCHUNK - 1, NG - 2)
                # rwb is consumed only 2.4us after rwa under the sc-outer
                # order, so load it as two halves to avoid exposing its DMA
                rwb = wfpool.tile([P, GL * F1], F16, tag="wf1")
                rwb_v = rwb[:].rearrange("p (l f) -> p l f", l=GL)
                HG = GL // 2
                nc.sync.dma_start(rwb_v[:, 0:HG], wf1_d.ap()[NCHUNK - 1, NG - 1, :, 0:HG])
                nc.sync.dma_start(rwb_v[:, HG:GL], wf1_d.ap()[NCHUNK - 1, NG - 1, :, HG:GL])
                for sc in range(SC):
                    for lg, rw in ((NG - 2, rwa), (NG - 1, rwb)):
                        for ll in range(GL):
                            fc1_mm(NCHUNK - 1, lg, ll, sc, rw)
                # FC2: elementwise multiply + free-dim reduce per sample chunk;
                # transpose the per-partition results to rows so the final
                # DMA is 2 contiguous descriptors (a [128,1] DMA's 256
                # spaced sem-incs would stall kernel teardown by ~7us)
                yc = h4pool.tile([P, SC], F32, tag="yc", name="yc")
                for sc in range(SC):
                    h = h4pool.tile([P, F1], F16, tag=f"h4_{sc}", name=f"h4_{sc}")
                    nc.scalar.activation(h[:], ps_fc1[sc][:], RELU)
                    scr = h4pool.tile([P, F1], F16, tag=f"fc2scr{sc}")
                    nc.vector.tensor_tensor(scr[:], h[:], wf2rep[:],
                                            mybir.AluOpType.mult)
                    y0 = h4pool.tile([P, 1], F32, tag=f"y0_{sc}", name=f"y0_{sc}")
                    nc.vector.tensor_reduce(y0[:], scr[:], mybir.AxisListType.X,
                                            mybir.AluOpType.add)
                    nc.vector.tensor_scalar_add(yc[:, sc:sc + 1], y0[:],
                                                bf2col[:])
                ytp = ypspool.tile([SC, P], F32, tag="ytp", bufs=1)
                nc.tensor.transpose(ytp[:], yc[:], ident[:])
                ystage = h4pool.tile([SC, P], F32, tag="ystage")
                nc.vector.tensor_copy(ystage[:], ytp[:])
                # trigger from the scalar engine (idle after the h4 relus):
                # fires right after the last copy instead of round-tripping
                # through the busy sync queue
                nc.scalar.dma_start(
                    y_d.ap().rearrange("(t b) one -> t b", t=SC), ystage[:])

    nc.compile()
    return nc


_NC_CACHE = None


def _get_nc():
    global _NC_CACHE
    if _NC_CACHE is None:
        _NC_CACHE = build_nc()
    return _NC_CACHE


def prep_inputs(x, Wp, bp, W1, b1, W2, b2, W3, b3, Wf1, bf1, Wf2, bf2):
    """Host-side shard + weight re-layout. Returns per-core input maps."""
    f32, f16 = np.float32, np.float16
    wp = np.asarray(Wp, f32)
    wpc = np.ascontiguousarray(
        np.concatenate([wp[:, :, 1].T, wp[:, :, 0].T], axis=0)).astype(f16)
    bpc = np.ascontiguousarray(np.asarray(bp, f32).reshape(NCHUNK, P))

    def conv_t(W):
        # W [co, ci, k] -> [cic, k, coc, ci, co]
        a = np.asarray(W, f32).reshape(NCHUNK, P, NCHUNK, P, 3)
        return np.ascontiguousarray(a.transpose(2, 4, 0, 3, 1)).astype(f16)

    w1t, w2t, w3t = conv_t(W1), conv_t(W2), conv_t(W3)
    b1c = np.ascontiguousarray(np.asarray(b1, f32).reshape(NCHUNK, P))
    b2c = np.ascontiguousarray(np.asarray(b2, f32).reshape(NCHUNK, P))
    b3c = np.ascontiguousarray(np.asarray(b3, f32).reshape(NCHUNK, P))
    # Wf1 [512, 30976] -> [cc, lg, c, ll, f] fp16 (contiguous per partition)
    wf1t = np.ascontiguousarray(
        np.asarray(Wf1, f32).reshape(F1, NCHUNK, P, LF)
        .transpose(1, 3, 2, 0)                        # [cc, l, c, f]
        .reshape(NCHUNK, LF // GL, GL, P, F1)
        .transpose(0, 1, 3, 2, 4)).astype(f16)        # [cc, lg, c, ll, f]
    bf1r = np.ascontiguousarray(np.asarray(bf1, f32).reshape(1, F1)).astype(f16)
    onesr = np.ones((1, P), f16)
    wf2rep = np.ascontiguousarray(
        np.broadcast_to(np.asarray(Wf2, f32).reshape(1, F1), (P, F1))).astype(f16)
    bf2col = np.ascontiguousarray(
        np.broadcast_to(np.asarray(bf2, f32).reshape(1, 1), (P, 1)))

    shared = dict(wpc=wpc, bpc=bpc, w1t=w1t, w2t=w2t, w3t=w3t,
                  b1c=b1c, b2c=b2c, b3c=b3c, wf1t=wf1t, bf1r=bf1r,
                  wf2rep=wf2rep, bf2col=bf2col, onesr=onesr)
    # x [B, CL*IL] -> per-core [j, b, i] (contiguous 256B per (j, b) line)
    xr = np.asarray(x, f32).reshape(N_CORES, BCORE, CL, IL).astype(f16)
    xth = np.ascontiguousarray(xr.transpose(0, 3, 1, 2))            # [nc, j, b, i]
    xbh = np.ascontiguousarray(
        np.broadcast_to(xth[:, :, :, 0:1], xth.shape))              # x0 repl over i
    return [dict(xth=xth[i], xbh=xbh[i], **shared) for i in range(N_CORES)]


def kernel(x, Wp, bp, W1, b1, W2, b2, W3, b3, Wf1, bf1, Wf2, bf2,
           trace=False, **run_kwargs):
    nc = _get_nc()
    in_maps = prep_inputs(x, Wp, bp, W1, b1, W2, b2, W3, b3, Wf1, bf1, Wf2, bf2)
    res = run_bass_kernel_spmd(nc, in_maps, core_ids=list(range(N_CORES)),
                               trace=trace, **run_kwargs)
    out = np.concatenate([res.results[i]["y"] for i in range(N_CORES)], axis=0)
    kernel.last_results = res
    return out.astype(np.float32)


kernel.last_results = None
